# revision 37
# baseline (speedup 1.0000x reference)
"""AttentionCell (Bahdanau attention + LSTM step) on 8 TRN2 NeuronCores.

Data-parallel over batch: B=256 rows sharded 32/core. Weights replicated.

Math per batch row b (T=256, IN=512, H=512, NE=96):
  proj_H  = batch_H @ W_i2h.T                       [T, H]
  proj_p  = prev_h @ W_h2h.T + b_h2h                [H]   (host-computed)
  e       = tanh(proj_H + proj_p) @ W_score[0]      [T]
  alpha   = softmax(e)                              [T]
  context = alpha @ batch_H                         [IN]
  gates   = context-part (device) + [onehot,1,prev_h]-part (host)
  i,f,g,o = split(gates); new_c = sig(f)*prev_c + sig(i)*tanh(g)
  new_h   = sig(o)*tanh(new_c)

Device pipeline, fully streamed per row-PAIR (2 rows fused, N=512 matmuls):
  PE:  proj^T (lhsT = W_i2h^T tiles, rhs = batch_H^T tiles, fp32 PSUM)
       -> e (lhsT = W_score chunks, rhs = tanh tiles)
  ACT: tanh with host-computed proj_prev^T as the per-partition bias;
       exp of the e row straight out of PSUM (e is bounded, no max-sub)
  DVE: softmax sum/recip/normalize; context^T = reduce_T(batch_H^T *
       alpha_bcast), written directly as context^T columns
  DMA: alpha rows to DRAM (doubles as output), then a stride-0 re-read
       broadcasts them across 128 partitions for the DVE context stage.
  The context stage for pair p is emitted two pairs late (and before the
  softmax ops of the current pair) so neither PE nor DVE ever stalls on
  the softmax round trip. The LSTM gates + elementwise tail run in two
  batch halves (32-aligned partition bases) right as each half of the
  context columns completes. Contraction chunks are 4-way interleaved
  (row 4p+k <-> partition p, chunk k) so every stream DMA moves 4KB
  contiguous per partition.
Matmul operands bf16 (fp32 accumulation); everything else fp32.
"""

import sys

sys.path.insert(0, "/opt/trn_rl_repo")

from contextlib import ExitStack

import ml_dtypes
import numpy as np

import concourse.bacc as bacc
import concourse.mybir as mybir
from concourse.bass_utils import run_bass_kernel_spmd
from concourse.tile import TileContext

F32 = mybir.dt.float32
BF16 = mybir.dt.bfloat16
AF = mybir.ActivationFunctionType
ALU = mybir.AluOpType
AX = mybir.AxisListType

B, T, IN, H, NE = 256, 256, 512, 512, 96
NCORES = 8
S = B // NCORES          # 32 batch rows per core
NP = S // 2              # 16 row-pairs per core
KI = IN // 128           # 4 interleaved contraction chunks over IN
KH = H // 128            # 4 chunks over H (contiguous, output side)

_bf16 = ml_dtypes.bfloat16


def _build():
    nc = bacc.Bacc("TRN2", target_bir_lowering=False, debug=False,
                   num_devices=NCORES)
    d = {
        "bht":    nc.dram_tensor("bht", [NP, IN, 512], BF16, kind="ExternalInput"),
        "wi2ht":  nc.dram_tensor("wi2ht", [IN, H], BF16, kind="ExternalInput"),
        "wscore": nc.dram_tensor("wscore", [128, KH], BF16, kind="ExternalInput"),
        "ppt":    nc.dram_tensor("ppt", [128, KH, S], F32, kind="ExternalInput"),
        "gpre":   nc.dram_tensor("gpre", [64, 4, 512], F32, kind="ExternalInput"),
        "wihtc":  nc.dram_tensor("wihtc", [4, IN, 512], BF16, kind="ExternalInput"),
        "prevc":  nc.dram_tensor("prevc", [64, H], F32, kind="ExternalInput"),
        "newh":   nc.dram_tensor("newh", [S, H], F32, kind="ExternalOutput"),
        "newc":   nc.dram_tensor("newc", [S, H], F32, kind="ExternalOutput"),
        # bf16: doubles as the alpha-broadcast DRAM scratch; host converts
        "alpha":  nc.dram_tensor("alpha", [S, T], BF16, kind="ExternalOutput"),
    }

    with TileContext(nc) as tc, ExitStack() as ctx:
        const = ctx.enter_context(tc.tile_pool(name="const", bufs=1))

        with tc.tile_pool(name="bhtP", bufs=NP) as bhtP, \
             tc.tile_pool(name="thP", bufs=3) as thP, \
             tc.tile_pool(name="smP", bufs=4) as smP, \
             tc.tile_pool(name="bcP", bufs=6) as bcP, \
             tc.tile_pool(name="tmpP", bufs=4) as tmpP, \
             tc.tile_pool(name="wG", bufs=4) as wG, \
             tc.tile_pool(name="lst", bufs=1) as lst:
            psCtx = ExitStack()
            psB = psCtx.enter_context(
                tc.tile_pool(name="psB", bufs=4, space="PSUM"))
            psE = psCtx.enter_context(
                tc.tile_pool(name="psE", bufs=2, space="PSUM"))

            bts = {}
            bc16s = {}
            wgs = []

            def bht_fetch(p):
                bt = bhtP.tile([128, KI, 512], BF16, tag="bht")
                bts[p] = bt
                nc.sync.dma_start(
                    out=bt[:],
                    in_=d["bht"].ap()[p].rearrange("(p k) x -> p k x", k=KI))

            # data for pair 0 first, then shared tensors, then the rest
            bht_fetch(0)
            wi2ht = const.tile([128, KI, H], BF16)
            nc.sync.dma_start(out=wi2ht[:],
                              in_=d["wi2ht"].ap().rearrange("(p k) h -> p k h",
                                                            k=KI))
            ppt = const.tile([128, KH, S], F32)
            nc.sync.dma_start(out=ppt[:], in_=d["ppt"].ap()[:])
            wsc = const.tile([128, KH], BF16)
            nc.sync.dma_start(out=wsc[:], in_=d["wscore"].ap()[:])
            bht_fetch(1)
            bht_fetch(2)
            zcol = const.tile([1, 1], F32)
            nc.gpsimd.memset(zcol[:], 0.0)
            gpre = const.tile([64, 4, 512], F32)
            nc.sync.dma_start(out=gpre[:], in_=d["gpre"].ap()[:])
            pc_sb = const.tile([64, H], F32)
            nc.sync.dma_start(out=pc_sb[:], in_=d["prevc"].ap()[:])

            ctxt = const.tile([128, KI, S], BF16)  # context^T accumulator

            def front(p):
                bt = bts[p]
                ths = []
                for m in range(KH):
                    ps = psB.tile([128, 512], F32, tag="pj")
                    for k in range(KI):
                        nc.tensor.matmul(ps[:],
                                         wi2ht[:, k, m * 128:(m + 1) * 128],
                                         bt[:, k, :],
                                         start=(k == 0), stop=(k == KI - 1))
                    th = thP.tile([128, 512], BF16, tag=f"th{m}")
                    for h in range(2):
                        bidx = 2 * p + h
                        nc.scalar.activation(th[:, h * 256:(h + 1) * 256],
                                             ps[:, h * 256:(h + 1) * 256],
                                             AF.Tanh,
                                             bias=ppt[:, m, bidx:bidx + 1],
                                             scale=1.0)
                    ths.append(th)
                pe = psE.tile([1, 512], F32, tag="e")
                for m in range(KH):
                    nc.tensor.matmul(pe[:], wsc[:, m:m + 1], ths[m][:],
                                     start=(m == 0), stop=(m == KH - 1))
                # softmax on the [1, 512] e row (2 rows side by side); e is
                # bounded so exp without max-subtraction is safe in fp32
                expr = smP.tile([1, 512], F32, tag="expr")
                nc.scalar.activation(expr[:], pe[:], AF.Exp, bias=zcol[:],
                                     scale=1.0)
                esum = smP.tile([1, 2], F32, tag="esum")
                nc.vector.tensor_reduce(
                    esum[:], expr[:].rearrange("o (h t) -> o h t", h=2),
                    axis=AX.X, op=ALU.add)
                rsum = smP.tile([1, 2], F32, tag="rsum")
                nc.vector.reciprocal(rsum[:], esum[:])
                anb = smP.tile([1, 512], BF16, tag="anb")
                nc.gpsimd.tensor_mul(
                    anb[:].rearrange("o (h t) -> o h t", h=2),
                    expr[:].rearrange("o (h t) -> o h t", h=2),
                    rsum[:].rearrange("o h -> o h ()").to_broadcast((1, 2, 256)))
                # normalized alpha rows -> DRAM output (also broadcast source)
                nc.gpsimd.dma_start(out=d["alpha"].ap()[2 * p:2 * p + 2, :],
                                    in_=anb[:])

            def bc_fetch(q):
                # stride-0 re-read replicating the alpha pair to 128 partitions
                bc16 = bcP.tile([128, 512], BF16, tag="bc16")
                bc16s[q] = bc16
                nc.sync.dma_start(
                    out=bc16[:],
                    in_=d["alpha"].ap()[2 * q:2 * q + 2, :]
                        .rearrange("(o h) t -> o (h t)", o=1)
                        .to_broadcast((128, 512)))

            def back(q):
                bt, bc16 = bts.pop(q), bc16s.pop(q)
                # context^T: multiply all IN-chunks by alpha, reduce over T
                tmp = tmpP.tile([128, KI, 512], BF16, tag="ctmp")
                nc.vector.tensor_mul(
                    tmp[:], bt[:],
                    bc16[:].rearrange("p (o x) -> p o x", o=1)
                         .to_broadcast((128, KI, 512)))
                with nc.allow_low_precision("bf16 ctx accum (fp32 internal)"):
                    nc.vector.tensor_reduce(
                        ctxt[:, :, 2 * q:2 * q + 2],
                        tmp[:].rearrange("p k (h t) -> p k h t", h=2),
                        axis=AX.X, op=ALU.add)

            newc_sb = lst.tile([64, H], F32, tag="newc")
            newh_sb = lst.tile([64, H], F32, tag="newh")

            def gates_half(hb):
                # LSTM for batch rows [16*hb, 16*hb+16): runs as soon as the
                # first/second half of the context columns is complete.
                # All row-indexed tiles are [64, ...] sliced at 32*hb so every
                # 2-input op sees equal, 32-aligned base partitions.
                r0 = 16 * hb
                a0, a1 = 32 * hb, 32 * hb + 16

                gate_sb = []
                for n in range(4):
                    ps_g = psE.tile([64, 512], F32, tag="g")
                    for k in range(KI):
                        nc.tensor.matmul(ps_g[a0:a1, :], ctxt[:, k, r0:r0 + 16],
                                         wgs[n][:, k, :],
                                         start=(k == 0), stop=(k == KI - 1))
                    gs = lst.tile([64, 512], F32, tag=f"gs{n}{hb}")
                    nc.vector.tensor_add(gs[a0:a1, :], ps_g[a0:a1, :],
                                         gpre[a0:a1, n, :])
                    gate_sb.append(gs)
                i_s = lst.tile([64, 512], F32, tag=f"i{hb}")
                f_s = lst.tile([64, 512], F32, tag=f"f{hb}")
                g_t = lst.tile([64, 512], F32, tag=f"g{hb}")
                o_s = lst.tile([64, 512], F32, tag=f"o{hb}")
                nc.scalar.activation(i_s[a0:a1, :], gate_sb[0][a0:a1, :], AF.Sigmoid)
                nc.scalar.activation(f_s[a0:a1, :], gate_sb[1][a0:a1, :], AF.Sigmoid)
                nc.scalar.activation(g_t[a0:a1, :], gate_sb[2][a0:a1, :], AF.Tanh)
                nc.scalar.activation(o_s[a0:a1, :], gate_sb[3][a0:a1, :], AF.Sigmoid)
                t1 = lst.tile([64, 512], F32, tag=f"t1{hb}")
                t2 = lst.tile([64, 512], F32, tag=f"t2{hb}")
                nc.vector.tensor_mul(t1[a0:a1, :], f_s[a0:a1, :], pc_sb[a0:a1, :])
                nc.vector.tensor_mul(t2[a0:a1, :], i_s[a0:a1, :], g_t[a0:a1, :])
                nc.vector.tensor_add(newc_sb[a0:a1, :], t1[a0:a1, :], t2[a0:a1, :])
                tcn = lst.tile([64, 512], F32, tag=f"tc{hb}")
                nc.scalar.activation(tcn[a0:a1, :], newc_sb[a0:a1, :], AF.Tanh)
                nc.vector.tensor_mul(newh_sb[a0:a1, :], o_s[a0:a1, :],
                                     tcn[a0:a1, :])

            for p in range(NP + 2):
                if p >= 2:
                    back(p - 2)
                if p < NP:
                    front(p)
                if p + 3 < NP:
                    bht_fetch(p + 3)
                if 1 <= p <= NP:
                    bc_fetch(p - 1)
                if p == 2:
                    for n in range(4):
                        wg = wG.tile([128, KI, 512], BF16, tag="wg")
                        nc.sync.dma_start(
                            out=wg[:],
                            in_=d["wihtc"].ap()[n]
                                .rearrange("(p k) x -> p k x", k=KI))
                        wgs.append(wg)
                if p == NP:
                    gates_half(0)
                if p == NP + 1:
                    gates_half(1)

            for hb in range(2):
                r0, a0 = 16 * hb, 32 * hb
                nc.sync.dma_start(out=d["newc"].ap()[r0:r0 + 16, :],
                                  in_=newc_sb[a0:a0 + 16, :])
                nc.sync.dma_start(out=d["newh"].ap()[r0:r0 + 16, :],
                                  in_=newh_sb[a0:a0 + 16, :])
            psCtx.close()

    nc.compile()
    return nc


_NC_CACHE = None


def _get_nc():
    global _NC_CACHE
    if _NC_CACHE is None:
        _NC_CACHE = _build()
    return _NC_CACHE


def _prep_inputs(prev_h, prev_c, batch_H, char_onehots,
                 W_i2h, W_h2h, b_h2h, W_score, W_ih, W_hh, b_ih, b_hh):
    """Host-side sharding, layout transforms, and the small precomputations
    (proj_prev and the context-independent part of the LSTM gates)."""
    f32 = np.float32
    prev_h = np.asarray(prev_h, f32)
    bht_all = np.ascontiguousarray(
        batch_H.astype(_bf16).reshape(NCORES, NP, 2, T, IN)
        .transpose(0, 1, 4, 2, 3).reshape(NCORES, NP, IN, 512))
    pc4 = prev_c.astype(f32).reshape(NCORES, S, H)
    prevc_all = np.zeros((NCORES, 64, H), f32)
    prevc_all[:, 0:16] = pc4[:, 0:16]
    prevc_all[:, 32:48] = pc4[:, 16:32]

    # proj_prev^T with b_h2h folded in: [core, 128, KH, S]
    pp = prev_h @ W_h2h.T + b_h2h                     # [B, H]
    ppt_all = np.ascontiguousarray(
        pp.T.astype(f32).reshape(KH, 128, NCORES, S).transpose(2, 1, 0, 3))

    # gate pre-accumulation: [onehot] @ W_ih[:, IN:].T + b + prev_h @ W_hh.T
    gpre = (char_onehots @ W_ih[:, IN:].T + (b_ih + b_hh)
            + prev_h @ W_hh.T)                        # [B, 4H]
    g4 = gpre.astype(f32).reshape(NCORES, S, 4, 512)
    gpre_all = np.zeros((NCORES, 64, 4, 512), f32)
    gpre_all[:, 0:16] = g4[:, 0:16]
    gpre_all[:, 32:48] = g4[:, 16:32]

    wi2ht = np.ascontiguousarray(W_i2h.T).astype(_bf16)
    wscore = np.ascontiguousarray(W_score[0].reshape(KH, 128).T).astype(_bf16)
    wihtc = np.ascontiguousarray(
        W_ih[:, :IN].T.reshape(IN, 4, 512).transpose(1, 0, 2)).astype(_bf16)

    return [{
        "bht": np.ascontiguousarray(bht_all[c]),
        "wi2ht": wi2ht,
        "wscore": wscore,
        "ppt": np.ascontiguousarray(ppt_all[c]),
        "gpre": gpre_all[c],
        "wihtc": wihtc,
        "prevc": np.ascontiguousarray(prevc_all[c]),
    } for c in range(NCORES)]


def _run(inputs, trace=False):
    nc = _get_nc()
    in_maps = _prep_inputs(**{k: np.asarray(v) for k, v in inputs.items()})
    res = run_bass_kernel_spmd(nc, in_maps, core_ids=list(range(NCORES)),
                               trace=trace)
    new_h = np.concatenate([res.results[c]["newh"] for c in range(NCORES)], 0)
    new_c = np.concatenate([res.results[c]["newc"] for c in range(NCORES)], 0)
    alpha = np.concatenate([res.results[c]["alpha"] for c in range(NCORES)], 0)
    return (new_h.astype(np.float32), new_c.astype(np.float32),
            alpha.astype(np.float32)[:, :, None]), res


def kernel(**inputs):
    out, _ = _run(inputs, trace=False)
    return out


# revision 38
# speedup vs baseline: 1.0484x; 1.0484x over previous
"""AttentionCell (Bahdanau attention + LSTM step) on 8 TRN2 NeuronCores.

Data-parallel over batch: B=256 rows sharded 32/core. Weights replicated.

Math per batch row b (T=256, IN=512, H=512, NE=96):
  proj_H  = batch_H @ W_i2h.T                       [T, H]
  proj_p  = prev_h @ W_h2h.T + b_h2h                [H]   (host-computed)
  e       = tanh(proj_H + proj_p) @ W_score[0]      [T]
  alpha   = softmax(e)                              [T]
  context = alpha @ batch_H                         [IN]
  gates   = context-part (device) + [onehot,1,prev_h]-part (host)
  i,f,g,o = split(gates); new_c = sig(f)*prev_c + sig(i)*tanh(g)
  new_h   = sig(o)*tanh(new_c)

Device pipeline, fully streamed per row-PAIR (2 rows fused, N=512 matmuls):
  PE:  proj^T (lhsT = W_i2h^T tiles, rhs = batch_H^T tiles, fp32 PSUM)
       -> e (lhsT = W_score chunks, rhs = tanh tiles)
  ACT: tanh with host-computed proj_prev^T as the per-partition bias;
       exp of the e row straight out of PSUM (e is bounded, no max-sub)
  DVE: softmax sum/recip/normalize; context^T = reduce_T(batch_H^T *
       alpha_bcast), written directly as context^T columns
  DMA: alpha rows to DRAM (doubles as output), then a stride-0 re-read
       broadcasts them across 128 partitions for the DVE context stage.
  The context stage for pair p is emitted two pairs late (and before the
  softmax ops of the current pair) so neither PE nor DVE ever stalls on
  the softmax round trip. The LSTM gates + elementwise tail run in two
  batch halves (32-aligned partition bases) right as each half of the
  context columns completes. Contraction chunks are 4-way interleaved
  (row 4p+k <-> partition p, chunk k) so every stream DMA moves 4KB
  contiguous per partition.
Matmul operands bf16 (fp32 accumulation); everything else fp32.
"""

import sys

sys.path.insert(0, "/opt/trn_rl_repo")

from contextlib import ExitStack

import ml_dtypes
import numpy as np

import concourse.bacc as bacc
import concourse.mybir as mybir
from concourse.bass_utils import run_bass_kernel_spmd
from concourse.tile import TileContext

F32 = mybir.dt.float32
BF16 = mybir.dt.bfloat16
AF = mybir.ActivationFunctionType
ALU = mybir.AluOpType
AX = mybir.AxisListType

B, T, IN, H, NE = 256, 256, 512, 512, 96
NCORES = 8
S = B // NCORES          # 32 batch rows per core
NP = S // 2              # 16 row-pairs per core
KI = IN // 128           # 4 interleaved contraction chunks over IN
KH = H // 128            # 4 chunks over H (contiguous, output side)

_bf16 = ml_dtypes.bfloat16


def _build():
    nc = bacc.Bacc("TRN2", target_bir_lowering=False, debug=False,
                   num_devices=NCORES)
    d = {
        "bht":    nc.dram_tensor("bht", [NP, IN, 512], BF16, kind="ExternalInput"),
        "wi2ht":  nc.dram_tensor("wi2ht", [IN, H], BF16, kind="ExternalInput"),
        "wscore": nc.dram_tensor("wscore", [128, KH], BF16, kind="ExternalInput"),
        "ppt":    nc.dram_tensor("ppt", [128, KH, S], F32, kind="ExternalInput"),
        "gpre":   nc.dram_tensor("gpre", [64, 4, 512], F32, kind="ExternalInput"),
        "wihtc":  nc.dram_tensor("wihtc", [4, IN, 512], BF16, kind="ExternalInput"),
        "prevc":  nc.dram_tensor("prevc", [64, H], F32, kind="ExternalInput"),
        "newh":   nc.dram_tensor("newh", [S, H], F32, kind="ExternalOutput"),
        "newc":   nc.dram_tensor("newc", [S, H], F32, kind="ExternalOutput"),
        # bf16: doubles as the alpha-broadcast DRAM scratch; host converts
        "alpha":  nc.dram_tensor("alpha", [S, T], BF16, kind="ExternalOutput"),
    }

    with TileContext(nc) as tc, ExitStack() as ctx:
        const = ctx.enter_context(tc.tile_pool(name="const", bufs=1))

        with tc.tile_pool(name="bhtP", bufs=NP) as bhtP, \
             tc.tile_pool(name="thP", bufs=3) as thP, \
             tc.tile_pool(name="smP", bufs=4) as smP, \
             tc.tile_pool(name="bcP", bufs=6) as bcP, \
             tc.tile_pool(name="tmpP", bufs=4) as tmpP, \
             tc.tile_pool(name="wG", bufs=4) as wG, \
             tc.tile_pool(name="lst", bufs=1) as lst:
            psCtx = ExitStack()
            psB = psCtx.enter_context(
                tc.tile_pool(name="psB", bufs=4, space="PSUM"))
            psE = psCtx.enter_context(
                tc.tile_pool(name="psE", bufs=2, space="PSUM"))

            bts = {}
            bc16s = {}
            wgs = []

            def bht_fetch(p):
                bt = bhtP.tile([128, KI, 512], BF16, tag="bht")
                bts[p] = bt
                nc.sync.dma_start(
                    out=bt[:],
                    in_=d["bht"].ap()[p].rearrange("(p k) x -> p k x", k=KI))

            # data for pair 0 first, then shared tensors, then the rest
            bht_fetch(0)
            wi2ht = const.tile([128, KI, H], BF16)
            nc.sync.dma_start(out=wi2ht[:],
                              in_=d["wi2ht"].ap().rearrange("(p k) h -> p k h",
                                                            k=KI))
            ppt = const.tile([128, KH, S], F32)
            nc.sync.dma_start(out=ppt[:], in_=d["ppt"].ap()[:])
            wsc = const.tile([128, KH], BF16)
            nc.sync.dma_start(out=wsc[:], in_=d["wscore"].ap()[:])
            bht_fetch(1)
            bht_fetch(2)
            zcol = const.tile([1, 1], F32)
            nc.gpsimd.memset(zcol[:], 0.0)
            gpre = const.tile([64, 4, 512], F32)
            nc.sync.dma_start(out=gpre[:], in_=d["gpre"].ap()[:])
            pc_sb = const.tile([64, H], F32)
            nc.sync.dma_start(out=pc_sb[:], in_=d["prevc"].ap()[:])

            ctxt = const.tile([128, KI, S], BF16)  # context^T accumulator

            def front(p):
                bt = bts[p]
                ths = []
                for m in range(KH):
                    ps = psB.tile([128, 512], F32, tag="pj")
                    for k in range(KI):
                        nc.tensor.matmul(ps[:],
                                         wi2ht[:, k, m * 128:(m + 1) * 128],
                                         bt[:, k, :],
                                         start=(k == 0), stop=(k == KI - 1))
                    th = thP.tile([128, 512], BF16, tag=f"th{m}")
                    for h in range(2):
                        bidx = 2 * p + h
                        nc.scalar.activation(th[:, h * 256:(h + 1) * 256],
                                             ps[:, h * 256:(h + 1) * 256],
                                             AF.Tanh,
                                             bias=ppt[:, m, bidx:bidx + 1],
                                             scale=1.0)
                    ths.append(th)
                pe = psE.tile([1, 512], F32, tag="e")
                for m in range(KH):
                    nc.tensor.matmul(pe[:], wsc[:, m:m + 1], ths[m][:],
                                     start=(m == 0), stop=(m == KH - 1))
                # softmax on the [1, 512] e row (2 rows side by side); e is
                # bounded so exp without max-subtraction is safe in fp32
                expr = smP.tile([1, 512], F32, tag="expr")
                nc.scalar.activation(expr[:], pe[:], AF.Exp, bias=zcol[:],
                                     scale=1.0)
                esum = smP.tile([1, 2], F32, tag="esum")
                nc.vector.tensor_reduce(
                    esum[:], expr[:].rearrange("o (h t) -> o h t", h=2),
                    axis=AX.X, op=ALU.add)
                rsum = smP.tile([1, 2], F32, tag="rsum")
                nc.vector.reciprocal(rsum[:], esum[:])
                anb = smP.tile([1, 512], BF16, tag="anb")
                nc.vector.tensor_mul(
                    anb[:].rearrange("o (h t) -> o h t", h=2),
                    expr[:].rearrange("o (h t) -> o h t", h=2),
                    rsum[:].rearrange("o h -> o h ()").to_broadcast((1, 2, 256)))
                # normalized alpha rows -> DRAM output (also broadcast source)
                nc.gpsimd.dma_start(out=d["alpha"].ap()[2 * p:2 * p + 2, :],
                                    in_=anb[:])

            def bc_fetch(q):
                # stride-0 re-read replicating the alpha pair to 128 partitions
                bc16 = bcP.tile([128, 512], BF16, tag="bc16")
                bc16s[q] = bc16
                nc.sync.dma_start(
                    out=bc16[:],
                    in_=d["alpha"].ap()[2 * q:2 * q + 2, :]
                        .rearrange("(o h) t -> o (h t)", o=1)
                        .to_broadcast((128, 512)))

            def back(q):
                bt, bc16 = bts.pop(q), bc16s.pop(q)
                # context^T: multiply all IN-chunks by alpha, reduce over T
                tmp = tmpP.tile([128, KI, 512], BF16, tag="ctmp")
                nc.vector.tensor_mul(
                    tmp[:], bt[:],
                    bc16[:].rearrange("p (o x) -> p o x", o=1)
                         .to_broadcast((128, KI, 512)))
                with nc.allow_low_precision("bf16 ctx accum (fp32 internal)"):
                    nc.vector.tensor_reduce(
                        ctxt[:, :, 2 * q:2 * q + 2],
                        tmp[:].rearrange("p k (h t) -> p k h t", h=2),
                        axis=AX.X, op=ALU.add)

            newc_sb = lst.tile([64, H], F32, tag="newc")
            newh_sb = lst.tile([64, H], F32, tag="newh")

            def gates_half(hb):
                # LSTM for batch rows [16*hb, 16*hb+16): runs as soon as the
                # first/second half of the context columns is complete.
                # All row-indexed tiles are [64, ...] sliced at 32*hb so every
                # 2-input op sees equal, 32-aligned base partitions.
                r0 = 16 * hb
                a0, a1 = 32 * hb, 32 * hb + 16

                gate_sb = []
                for n in range(4):
                    ps_g = psE.tile([64, 512], F32, tag="g")
                    for k in range(KI):
                        nc.tensor.matmul(ps_g[a0:a1, :], ctxt[:, k, r0:r0 + 16],
                                         wgs[n][:, k, :],
                                         start=(k == 0), stop=(k == KI - 1))
                    gs = lst.tile([64, 512], F32, tag=f"gs{n}{hb}")
                    nc.vector.tensor_add(gs[a0:a1, :], ps_g[a0:a1, :],
                                         gpre[a0:a1, n, :])
                    gate_sb.append(gs)
                i_s = lst.tile([64, 512], F32, tag=f"i{hb}")
                f_s = lst.tile([64, 512], F32, tag=f"f{hb}")
                g_t = lst.tile([64, 512], F32, tag=f"g{hb}")
                o_s = lst.tile([64, 512], F32, tag=f"o{hb}")
                nc.scalar.activation(i_s[a0:a1, :], gate_sb[0][a0:a1, :], AF.Sigmoid)
                nc.scalar.activation(f_s[a0:a1, :], gate_sb[1][a0:a1, :], AF.Sigmoid)
                nc.scalar.activation(g_t[a0:a1, :], gate_sb[2][a0:a1, :], AF.Tanh)
                nc.scalar.activation(o_s[a0:a1, :], gate_sb[3][a0:a1, :], AF.Sigmoid)
                t1 = lst.tile([64, 512], F32, tag=f"t1{hb}")
                t2 = lst.tile([64, 512], F32, tag=f"t2{hb}")
                nc.vector.tensor_mul(t1[a0:a1, :], f_s[a0:a1, :], pc_sb[a0:a1, :])
                nc.vector.tensor_mul(t2[a0:a1, :], i_s[a0:a1, :], g_t[a0:a1, :])
                nc.vector.tensor_add(newc_sb[a0:a1, :], t1[a0:a1, :], t2[a0:a1, :])
                tcn = lst.tile([64, 512], F32, tag=f"tc{hb}")
                nc.scalar.activation(tcn[a0:a1, :], newc_sb[a0:a1, :], AF.Tanh)
                nc.vector.tensor_mul(newh_sb[a0:a1, :], o_s[a0:a1, :],
                                     tcn[a0:a1, :])

            for p in range(NP + 2):
                if p >= 2:
                    back(p - 2)
                if p < NP:
                    front(p)
                if p + 3 < NP:
                    bht_fetch(p + 3)
                if 1 <= p <= NP:
                    bc_fetch(p - 1)
                if p == 2:
                    for n in range(4):
                        wg = wG.tile([128, KI, 512], BF16, tag="wg")
                        nc.sync.dma_start(
                            out=wg[:],
                            in_=d["wihtc"].ap()[n]
                                .rearrange("(p k) x -> p k x", k=KI))
                        wgs.append(wg)
                if p == NP:
                    gates_half(0)
                if p == NP + 1:
                    gates_half(1)

            for hb in range(2):
                r0, a0 = 16 * hb, 32 * hb
                nc.sync.dma_start(out=d["newc"].ap()[r0:r0 + 16, :],
                                  in_=newc_sb[a0:a0 + 16, :])
                nc.sync.dma_start(out=d["newh"].ap()[r0:r0 + 16, :],
                                  in_=newh_sb[a0:a0 + 16, :])
            psCtx.close()

    nc.compile()
    return nc


_NC_CACHE = None


def _get_nc():
    global _NC_CACHE
    if _NC_CACHE is None:
        _NC_CACHE = _build()
    return _NC_CACHE


def _prep_inputs(prev_h, prev_c, batch_H, char_onehots,
                 W_i2h, W_h2h, b_h2h, W_score, W_ih, W_hh, b_ih, b_hh):
    """Host-side sharding, layout transforms, and the small precomputations
    (proj_prev and the context-independent part of the LSTM gates)."""
    f32 = np.float32
    prev_h = np.asarray(prev_h, f32)
    bht_all = np.ascontiguousarray(
        batch_H.astype(_bf16).reshape(NCORES, NP, 2, T, IN)
        .transpose(0, 1, 4, 2, 3).reshape(NCORES, NP, IN, 512))
    pc4 = prev_c.astype(f32).reshape(NCORES, S, H)
    prevc_all = np.zeros((NCORES, 64, H), f32)
    prevc_all[:, 0:16] = pc4[:, 0:16]
    prevc_all[:, 32:48] = pc4[:, 16:32]

    # proj_prev^T with b_h2h folded in: [core, 128, KH, S]
    pp = prev_h @ W_h2h.T + b_h2h                     # [B, H]
    ppt_all = np.ascontiguousarray(
        pp.T.astype(f32).reshape(KH, 128, NCORES, S).transpose(2, 1, 0, 3))

    # gate pre-accumulation: [onehot] @ W_ih[:, IN:].T + b + prev_h @ W_hh.T
    gpre = (char_onehots @ W_ih[:, IN:].T + (b_ih + b_hh)
            + prev_h @ W_hh.T)                        # [B, 4H]
    g4 = gpre.astype(f32).reshape(NCORES, S, 4, 512)
    gpre_all = np.zeros((NCORES, 64, 4, 512), f32)
    gpre_all[:, 0:16] = g4[:, 0:16]
    gpre_all[:, 32:48] = g4[:, 16:32]

    wi2ht = np.ascontiguousarray(W_i2h.T).astype(_bf16)
    wscore = np.ascontiguousarray(W_score[0].reshape(KH, 128).T).astype(_bf16)
    wihtc = np.ascontiguousarray(
        W_ih[:, :IN].T.reshape(IN, 4, 512).transpose(1, 0, 2)).astype(_bf16)

    return [{
        "bht": np.ascontiguousarray(bht_all[c]),
        "wi2ht": wi2ht,
        "wscore": wscore,
        "ppt": np.ascontiguousarray(ppt_all[c]),
        "gpre": gpre_all[c],
        "wihtc": wihtc,
        "prevc": np.ascontiguousarray(prevc_all[c]),
    } for c in range(NCORES)]


def _run(inputs, trace=False):
    nc = _get_nc()
    in_maps = _prep_inputs(**{k: np.asarray(v) for k, v in inputs.items()})
    res = run_bass_kernel_spmd(nc, in_maps, core_ids=list(range(NCORES)),
                               trace=trace)
    new_h = np.concatenate([res.results[c]["newh"] for c in range(NCORES)], 0)
    new_c = np.concatenate([res.results[c]["newc"] for c in range(NCORES)], 0)
    alpha = np.concatenate([res.results[c]["alpha"] for c in range(NCORES)], 0)
    return (new_h.astype(np.float32), new_c.astype(np.float32),
            alpha.astype(np.float32)[:, :, None]), res


def kernel(**inputs):
    out, _ = _run(inputs, trace=False)
    return out
